# revision 26
# baseline (speedup 1.0000x reference)
"""Distributed Trainium2 kernel for nn_Attention_37958920962105.

GQA attention layer (DIM=4096, 32 q heads, 8 kv heads, head_dim=128,
B=2, S=2048) sharded tensor-parallel over GQA groups across 8 cores:
core c owns q heads 4c..4c+3 and kv head c.  Per core:
  1. QKV projection (transposed layouts) + RoPE + v transpose
  2. attention (scores -> exp -> ones-matmul denominators -> AV)
  3. AllToAll (one per local head) to token-shard y
  4. out projection on this core's 512-token chunk
Output chunks are reassembled on the host.
"""

import math
import sys
import types
from contextlib import ExitStack

import numpy as np
import ml_dtypes

import concourse.bass as bass
import concourse.mybir as mybir
import concourse.tile as tile
from concourse import bacc
from concourse.bass_utils import run_bass_kernel_spmd

BF = mybir.dt.bfloat16
F32 = mybir.dt.float32
bf16 = ml_dtypes.bfloat16

P = 128
DIM = 4096
N_HEAD = 32
N_KV = 8
HD = 128
B = 2
S = 2048
TOK = B * S          # 4096
NCORES = 8
HPC = N_HEAD // N_KV  # 4 q heads per core
FQKV = (HPC + 2) * HD  # 768 qkv rows per core
KC = DIM // P        # 32 contraction chunks
N_TT = TOK // 512    # 8 token tiles of 512
QT_N = S // 512      # 4 q tiles per batch
KT_N = S // P        # 16 k tiles per batch
SCALE = 1.0 / math.sqrt(HD)


def _install_profile_shim():
    if 'antenv.axon_hooks' in sys.modules:
        return
    try:
        from trn_agent_boot.trn_boot import _ntff_profile_via_ctypes
        hook = _ntff_profile_via_ctypes('/opt/axon/libaxon_pjrt.so')
    except Exception:
        hook = None
    mod = types.ModuleType('antenv.axon_hooks')
    mod._hook = hook
    mod.get_axon_ntff_profile_hook = lambda: mod._hook
    mod.set_axon_ntff_profile_hook = lambda h: setattr(mod, '_hook', h)
    sys.modules['antenv.axon_hooks'] = mod
    try:
        import antenv
        antenv.axon_hooks = mod
    except ImportError:
        pass


# ---------------------------------------------------------------------------
# host-side prep
# ---------------------------------------------------------------------------

def _classify_mask(mask):
    """mask: [S(q), S(k)] bool.  Returns (cls, mask_tiles) where
    cls[qt][kt] in {'skip', 'free', int mask-tile-index}; mask tiles are
    transposed [128 k, 512 q] bf16."""
    cls = [[None] * KT_N for _ in range(QT_N)]
    tiles = []
    seen = {}
    qi = np.arange(512)[:, None]
    ki = np.arange(P)[None, :]
    for qt in range(QT_N):
        for kt in range(KT_N):
            blk = mask[qt * 512:(qt + 1) * 512, kt * P:(kt + 1) * P]
            if not blk.any():
                cls[qt][kt] = 'skip'
                continue
            if blk.all():
                cls[qt][kt] = 'free'
                continue
            off = kt * P - qt * 512
            if 0 <= off < 512 and np.array_equal(blk, (off + ki) <= qi):
                # causal diagonal tile: valid only for q >= off, and within
                # the first 128 valid q columns it is the k<=q' triangle
                cls[qt][kt] = ('diag', off)
                continue
            key = blk.tobytes()
            if key not in seen:
                seen[key] = len(tiles)
                tiles.append(np.ascontiguousarray(blk.T).astype(bf16))
            cls[qt][kt] = seen[key]
    return cls, tiles


def _prep(x, freqs_cis, mask_cache, wqkv, wo):
    x = np.asarray(x, dtype=np.float32)
    freqs_cis = np.asarray(freqs_cis, dtype=np.float32)
    wqkv = np.asarray(wqkv, dtype=np.float32)
    wo = np.asarray(wo, dtype=np.float32)
    mask = np.asarray(mask_cache)[0, 0]

    xT = np.ascontiguousarray(x.reshape(TOK, DIM).T).astype(bf16)

    wTs = []
    for c in range(NCORES):
        w_c = np.concatenate([
            wqkv[HPC * HD * c: HPC * HD * (c + 1)],          # 4 q heads
            wqkv[N_HEAD * HD + HD * c: N_HEAD * HD + HD * (c + 1)],   # k head
            wqkv[(N_HEAD + N_KV) * HD + HD * c:
                 (N_HEAD + N_KV) * HD + HD * (c + 1)],       # v head
        ], axis=0)                                           # [768, DIM]
        wTs.append(np.ascontiguousarray(w_c.T).astype(bf16))  # [DIM, 768]

    # wo permuted so row-block dbi = m*8 + cc holds global head 4*cc + m
    woT = np.ascontiguousarray(wo.T)                 # [d, o]
    woT_h = woT.reshape(N_HEAD, HD, DIM)
    perm = [4 * (dbi % NCORES) + dbi // NCORES for dbi in range(N_HEAD)]
    woT_perm = np.ascontiguousarray(woT_h[perm].reshape(DIM, DIM)).astype(bf16)

    f0 = freqs_cis[:, :, 0].T                        # [64, S]
    f1 = freqs_cis[:, :, 1].T
    ropeA = np.repeat(f0, 2, axis=0).astype(bf16)    # [128, S]
    ropeB = np.empty((HD, S), dtype=np.float32)
    ropeB[0::2] = -f1
    ropeB[1::2] = f1
    ropeB = ropeB.astype(bf16)

    pswap = np.zeros((P, P), dtype=bf16)
    for i in range(P):
        pswap[i, i ^ 1] = 1
    ident = np.eye(P, dtype=bf16)
    ones_col = np.ones((P, 1), dtype=bf16)
    ones_row = np.ones((1, P), dtype=bf16)
    onesc_f32 = np.ones((1, P), dtype=np.float32)

    tri = (np.arange(P)[:, None] <= np.arange(P)[None, :]).astype(bf16)
    cls, mask_tiles = _classify_mask(mask)
    masks = (np.concatenate([t for t in mask_tiles], axis=0)
             if mask_tiles else None)                # [n*128, 512] bf16

    return dict(xT=xT, wTs=wTs, woT=woT_perm, ropeA=ropeA, ropeB=ropeB,
                pswap=pswap, ident=ident, ones=ones_col, onesr=ones_row,
                onesc=onesc_f32, tri=tri, cls=cls, masks=masks)


# ---------------------------------------------------------------------------
# device kernel builder
# ---------------------------------------------------------------------------

def _build(cls, n_masks, debug=False):
    nc = bacc.Bacc("TRN2", target_bir_lowering=False, debug=False,
                   num_devices=NCORES)
    xT_d = nc.dram_tensor("xT", [DIM, TOK], BF, kind="ExternalInput")
    wT_d = nc.dram_tensor("wT", [DIM, FQKV], BF, kind="ExternalInput")
    woT_d = nc.dram_tensor("woT", [DIM, DIM], BF, kind="ExternalInput")
    ropeA_d = nc.dram_tensor("ropeA", [P, S], BF, kind="ExternalInput")
    ropeB_d = nc.dram_tensor("ropeB", [P, S], BF, kind="ExternalInput")
    pswap_d = nc.dram_tensor("pswap", [P, P], BF, kind="ExternalInput")
    ident_d = nc.dram_tensor("ident", [P, P], BF, kind="ExternalInput")
    ones_d = nc.dram_tensor("ones", [P, 1], BF, kind="ExternalInput")
    onesr_d = nc.dram_tensor("onesr", [1, P], BF, kind="ExternalInput")
    tri_d = nc.dram_tensor("tri", [P, P], BF, kind="ExternalInput")
    masks_d = (nc.dram_tensor("masks", [n_masks * P, 512], BF,
                              kind="ExternalInput") if n_masks else None)
    out_d = nc.dram_tensor("out", [512, DIM], BF, kind="ExternalOutput")
    if debug:
        dbg_qT0 = nc.dram_tensor("dbg_qT0", [P, TOK], BF, kind="ExternalOutput")
        dbg_kT = nc.dram_tensor("dbg_kT", [P, TOK], BF, kind="ExternalOutput")
        dbg_v = nc.dram_tensor("dbg_v", [P, TOK], BF, kind="ExternalOutput")
        dbg_a2ai = nc.dram_tensor("dbg_a2ai", [8 * P, 512], BF, kind="ExternalOutput")
        dbg_a2ao = nc.dram_tensor("dbg_a2ao", [8 * P, 512], BF, kind="ExternalOutput")

    EXP = mybir.ActivationFunctionType.Exp
    rg = [list(range(NCORES))]

    with tile.TileContext(nc) as tc:
        with ExitStack() as top:
            const = top.enter_context(tc.tile_pool(name="const", bufs=1))
            acts = top.enter_context(tc.tile_pool(name="acts", bufs=1))
            dramp = top.enter_context(tc.tile_pool(name="dramp", bufs=1,
                                                   space="DRAM"))

            qT = [acts.tile([P, TOK], BF, name=f"qT{h}") for h in range(HPC)]
            kT = acts.tile([P, TOK], BF, name="kT")
            vv = [acts.tile([P, P], BF, name=f"v{i}") for i in range(TOK // P)]
            yf = [acts.tile([P, 512], BF, name=f"yf{i}") for i in range(KC)]

            a2a_in = [dramp.tile([NCORES * P, 512], BF, name=f"a2ai{m}")
                      for m in range(HPC)]
            a2a_out = [dramp.tile([NCORES * P, 512], BF, name=f"a2ao{m}")
                       for m in range(HPC)]

            # ---------------- phase 1: QKV + rope + v transpose ----------
            with ExitStack() as ph1, nc.named_scope("ph1_qkv"):
                wp = ph1.enter_context(tc.tile_pool(name="wp", bufs=1))
                xp = ph1.enter_context(tc.tile_pool(name="xp", bufs=32))
                w_sb = []
                xts0 = []
                for kc in range(KC):
                    w = wp.tile([P, FQKV], BF, name=f"w{kc}")
                    (nc.sync if kc % 2 else nc.scalar).dma_start(
                        w[:], wT_d[kc * P:(kc + 1) * P, :])
                    w_sb.append(w)
                    xt = xp.tile([P, 512], BF, name="xt")
                    (nc.scalar if kc % 2 else nc.sync).dma_start(
                        xt[:], xT_d[kc * P:(kc + 1) * P, 0:512])
                    xts0.append(xt)

                ropeA_sb = const.tile([P, S], BF, name="ropeA_sb")
                nc.sync.dma_start(ropeA_sb[:], ropeA_d[:])
                ropeB_sb = const.tile([P, S], BF, name="ropeB_sb")
                nc.sync.dma_start(ropeB_sb[:], ropeB_d[:])
                pswap_sb = const.tile([P, P], BF, name="pswap_sb")
                nc.sync.dma_start(pswap_sb[:], pswap_d[:])
                ident_sb = const.tile([P, P], BF, name="ident_sb")
                nc.sync.dma_start(ident_sb[:], ident_d[:])
                ones_sb = const.tile([P, 1], BF, name="ones_sb")
                nc.sync.dma_start(ones_sb[:], ones_d[:])
                onesr_sb = const.tile([1, P], BF, name="onesr_sb")
                nc.sync.dma_start(onesr_sb[:], onesr_d[:])
                tri_sb = const.tile([P, P], BF, name="tri_sb")
                nc.sync.dma_start(tri_sb[:], tri_d[:])
                mask_sb = []
                for i in range(n_masks):
                    m = const.tile([P, 512], BF, name=f"mask{i}")
                    nc.sync.dma_start(m[:], masks_d[i * P:(i + 1) * P, :])
                    mask_sb.append(m)
                qkvp = ph1.enter_context(
                    tc.tile_pool(name="qkvp", bufs=6, space="PSUM"))
                miscp = ph1.enter_context(
                    tc.tile_pool(name="miscp", bufs=2, space="PSUM"))
                stg = ph1.enter_context(tc.tile_pool(name="stg", bufs=4))

                for tt in range(N_TT):
                    s0 = (tt % QT_N) * 512
                    if tt == 0:
                        xts = xts0
                    else:
                        xts = []
                        for kc in range(KC):
                            xt = xp.tile([P, 512], BF, name="xt")
                            (nc.scalar if kc % 2 else nc.sync).dma_start(
                                xt[:], xT_d[kc * P:(kc + 1) * P,
                                            tt * 512:(tt + 1) * 512])
                            xts.append(xt)
                    pss = [qkvp.tile([P, 512], F32, name="qkvps")
                           for _ in range(6)]
                    for kc in range(KC):
                        for f in range(6):
                            nc.tensor.matmul(
                                pss[f][:], w_sb[kc][:, f * P:(f + 1) * P],
                                xts[kc][:], start=(kc == 0),
                                stop=(kc == KC - 1))
                    for f in range(6):
                        raw = stg.tile([P, 512], BF, name="raw")
                        nc.scalar.copy(raw[:], pss[f][:])
                        if f < 5:
                            swp = miscp.tile([P, 512], F32, name="miscps")
                            nc.tensor.matmul(swp[:], pswap_sb[:], raw[:],
                                             start=True, stop=True)
                            sw_sb = stg.tile([P, 512], BF, name="sw_sb")
                            nc.scalar.copy(sw_sb[:], swp[:])
                            r1 = stg.tile([P, 512], BF, name="r1")
                            nc.vector.tensor_mul(r1[:], raw[:],
                                                 ropeA_sb[:, s0:s0 + 512])
                            r2 = stg.tile([P, 512], BF, name="r2")
                            nc.vector.tensor_mul(r2[:], sw_sb[:],
                                                 ropeB_sb[:, s0:s0 + 512])
                            dst = (qT[f] if f < HPC else kT)
                            nc.vector.tensor_add(
                                dst[:, tt * 512:(tt + 1) * 512], r1[:], r2[:])
                        else:
                            for j in range(4):
                                tp = miscp.tile([P, 512], BF, name="miscps")
                                nc.tensor.transpose(
                                    tp[:, :P], raw[:, j * P:(j + 1) * P],
                                    ident_sb[:])
                                nc.vector.tensor_copy(vv[tt * 4 + j][:], tp[:, :P])

            # wo prefetch pool opened early so its DMAs (scalar ring) can
            # stream during attention
            wop = top.enter_context(tc.tile_pool(name="wop", bufs=2))
            wo_tiles = {}

            def wo_prefetch(ot, eng=None):
                t = wop.tile([P, KC, 512], BF, name="wo_sb")
                for dc in range(KC):
                    e = eng or (nc.scalar if dc % 2 else nc.sync)
                    e.dma_start(
                        t[:, dc, :],
                        woT_d[dc * P:(dc + 1) * P, ot * 512:(ot + 1) * 512])
                wo_tiles[ot] = t

            wo_prefetch(0, nc.sync)
            wo_prefetch(1, nc.sync)

            # ---------------- phase 2: attention + A2A -------------------
            with ExitStack() as ph2, nc.named_scope("ph2_attn"):
                sp = ph2.enter_context(
                    tc.tile_pool(name="sp", bufs=4, space="PSUM"))
                yp = ph2.enter_context(
                    tc.tile_pool(name="yp", bufs=2, space="PSUM"))
                dp = ph2.enter_context(
                    tc.tile_pool(name="dp", bufs=1, space="PSUM"))
                bp = ph2.enter_context(
                    tc.tile_pool(name="bp", bufs=1, space="PSUM"))
                ep = ph2.enter_context(tc.tile_pool(name="ep", bufs=6))
                eaccp = ph2.enter_context(tc.tile_pool(name="eaccp", bufs=2))
                ys = ph2.enter_context(tc.tile_pool(name="ys", bufs=5))
                rp = ph2.enter_context(tc.tile_pool(name="rp", bufs=4))

                def finish_norm(h, b, qt, yu_sb, e_acc):
                    # D = col-sums of e_acc via ones-matmul; broadcast 1/D
                    # across partitions with a rank-1 matmul (no DRAM trip)
                    ps_d = dp.tile([1, 512], F32, name="psd")
                    nc.tensor.matmul(ps_d[:], ones_sb[:], e_acc[:],
                                     start=True, stop=True)
                    rec = rp.tile([1, 512], F32, name="rec")
                    nc.vector.reciprocal_approx_fast(rec[:], ps_d[:])
                    rec_b = rp.tile([1, 512], BF, name="rec_b")
                    nc.vector.tensor_copy(rec_b[:], rec[:])
                    ps_b = bp.tile([P, 512], F32, name="psb")
                    nc.tensor.matmul(ps_b[:], onesr_sb[:], rec_b[:],
                                     start=True, stop=True)
                    y_sb = ys.tile([P, 512], BF, name="y_sb")
                    nc.vector.tensor_mul(y_sb[:], yu_sb[:], ps_b[:])
                    r = b * QT_N + qt
                    nc.sync.dma_start(
                        a2a_in[h][r * P:(r + 1) * P, :], y_sb[:])

                yf_pending = []

                def flush_yf(split=False):
                    while yf_pending:
                        hh = yf_pending.pop(0)
                        for cc in range(NCORES):
                            # scalar only helps on the final flush, when the
                            # exp stream is finished and scalar sits idle
                            eng = (nc.scalar if split and cc % 2
                                   else nc.gpsimd)
                            eng.dma_start(
                                yf[hh * NCORES + cc][:],
                                a2a_out[hh][cc * P:(cc + 1) * P, :])

                for h in range(HPC):
                    pending = []
                    flush_yf()
                    for b in range(B):
                        for qt in range(QT_N):
                            kts = [kt for kt in range(KT_N)
                                   if cls[qt][kt] != 'skip']
                            ps_y = yp.tile([P, 512], F32, name="psy")
                            e_acc = eaccp.tile([P, 512], BF, name="eacc")
                            for i, kt in enumerate(kts):
                                c = cls[qt][kt]
                                off = 0
                                tri_mask = None
                                if isinstance(c, tuple):
                                    off = c[1]
                                    tri_mask = tri_sb
                                w_q = 512 - off
                                q0 = b * S + qt * 512 + off
                                ps_s = sp.tile([P, 512], F32, name="pss")
                                nc.tensor.matmul(
                                    ps_s[:, :w_q],
                                    kT[:, b * S + kt * P: b * S + (kt + 1) * P],
                                    qT[h][:, bass.ds(q0, w_q)],
                                    start=True, stop=True)
                                # first tile (always full width) exps straight
                                # into the accumulator; later tiles add on DVE
                                if i == 0:
                                    e = e_acc
                                else:
                                    e = ep.tile([P, w_q], BF, name="e")
                                nc.scalar.activation(e[:], ps_s[:, :w_q], EXP,
                                                     scale=SCALE)
                                if tri_mask is not None:
                                    nc.vector.tensor_mul(
                                        e[:, :P], e[:, :P], tri_mask[:])
                                elif c != 'free':
                                    em = ep.tile([P, 512], BF, name="em")
                                    nc.vector.tensor_mul(
                                        em[:], e[:], mask_sb[c][:])
                                    e = em
                                if i > 0:
                                    nc.vector.tensor_add(
                                        e_acc[:, off:512],
                                        e_acc[:, off:512], e[:, :w_q])
                                st, sf = (i == 0), (i == len(kts) - 1)
                                nc.tensor.matmul(ps_y[:, off:512],
                                                 vv[b * KT_N + kt][:],
                                                 e[:], start=st, stop=sf,
                                                 skip_group_check=True)
                            yu_sb = ys.tile([P, 512], F32, name="yu_sb")
                            nc.vector.tensor_copy(yu_sb[:], ps_y[:])
                            pending.append((h, b, qt, yu_sb, e_acc))
                            if len(pending) > 1:
                                finish_norm(*pending.pop(0))
                    for pn in pending:
                        finish_norm(*pn)
                    pending = []
                    nc.gpsimd.collective_compute(
                        "AllToAll", mybir.AluOpType.bypass,
                        replica_groups=rg,
                        ins=[a2a_in[h].opt()], outs=[a2a_out[h].opt()])
                    yf_pending.append(h)
                flush_yf(split=True)

            # ---------------- phase 3: out projection --------------------
            with ExitStack() as ph3, nc.named_scope("ph3_outp"):
                opp = ph3.enter_context(
                    tc.tile_pool(name="opp", bufs=4, space="PSUM"))
                osb = ph3.enter_context(tc.tile_pool(name="osb", bufs=3))
                for ot in range(8):
                    wo_sb = wo_tiles.pop(ot)
                    if ot + 2 < 8:
                        wo_prefetch(ot + 2)
                    if ot == 0:
                        # first o-tile: run the first 24 d-chunks for every
                        # token subtile before touching m=3 blocks, so the
                        # last AllToAll's latency hides under real work
                        psos = [opp.tile([P, 512], F32, name="pso")
                                for _ in range(4)]
                        for ts in range(4):
                            for dc in range(24):
                                nc.tensor.matmul(
                                    psos[ts][:], yf[dc][:, ts * P:(ts + 1) * P],
                                    wo_sb[:, dc, :], start=(dc == 0),
                                    stop=False, skip_group_check=True)
                        for ts in range(4):
                            for dc in range(24, KC):
                                nc.tensor.matmul(
                                    psos[ts][:], yf[dc][:, ts * P:(ts + 1) * P],
                                    wo_sb[:, dc, :], start=False,
                                    stop=(dc == KC - 1), skip_group_check=True)
                            ob = osb.tile([P, 512], BF, name="ob")
                            nc.scalar.copy(ob[:], psos[ts][:])
                            nc.sync.dma_start(
                                out_d[ts * P:(ts + 1) * P, 0:512], ob[:])
                        continue
                    for ts in range(4):
                        pso = opp.tile([P, 512], F32, name="pso")
                        for dc in range(KC):
                            nc.tensor.matmul(
                                pso[:], yf[dc][:, ts * P:(ts + 1) * P],
                                wo_sb[:, dc, :], start=(dc == 0),
                                stop=(dc == KC - 1))
                        ob = osb.tile([P, 512], BF, name="ob")
                        if ot == 7 and ts % 2:
                            nc.vector.tensor_copy(ob[:], pso[:])
                        else:
                            nc.scalar.copy(ob[:], pso[:])
                        (nc.gpsimd if ot == 7 and ts % 2 else
                         nc.sync).dma_start(
                            out_d[ts * P:(ts + 1) * P,
                                  ot * 512:(ot + 1) * 512], ob[:])

            if debug:
                nc.sync.dma_start(dbg_qT0[:], qT[0][:])
                nc.sync.dma_start(dbg_kT[:], kT[:])
                for i in range(TOK // P):
                    nc.sync.dma_start(dbg_v[:, i * P:(i + 1) * P], vv[i][:])
                nc.sync.dma_start(dbg_a2ai[:], a2a_in[0][:])
                nc.sync.dma_start(dbg_a2ao[:], a2a_out[0][:])

    nc.compile()
    return nc


# ---------------------------------------------------------------------------
# public entry
# ---------------------------------------------------------------------------

_CACHE = {}


def _execute(x, freqs_cis, mask_cache, input_pos, wqkv, wo,
             trace=False, debug=False):
    _install_profile_shim()
    prep = _prep(x, freqs_cis, mask_cache, wqkv, wo)
    cls = prep['cls']
    n_masks = 0 if prep['masks'] is None else prep['masks'].shape[0] // P
    key = (str(cls), n_masks, debug)
    if key not in _CACHE:
        _CACHE[key] = _build(cls, n_masks, debug=debug)
    nc = _CACHE[key]

    in_maps = []
    for c in range(NCORES):
        m = dict(xT=prep['xT'], wT=prep['wTs'][c], woT=prep['woT'],
                 ropeA=prep['ropeA'], ropeB=prep['ropeB'],
                 pswap=prep['pswap'], ident=prep['ident'],
                 ones=prep['ones'], onesr=prep['onesr'], tri=prep['tri'])
        if n_masks:
            m['masks'] = prep['masks']
        in_maps.append(m)

    res = run_bass_kernel_spmd(nc, in_maps, core_ids=list(range(NCORES)),
                               trace=trace,
                               trace_cores=list(range(NCORES)) if trace
                               else None)
    out = np.zeros((B, S, DIM), dtype=np.float32)
    for c in range(NCORES):
        b, j = c // QT_N, c % QT_N
        out[b, j * 512:(j + 1) * 512] = \
            res.results[c]['out'].astype(np.float32)
    return out, res


def kernel(x, freqs_cis, mask_cache, input_pos, wqkv, wo):
    out, _ = _execute(x, freqs_cis, mask_cache, input_pos, wqkv, wo)
    return out


# ---------------------------------------------------------------------------
# numpy simulation of the exact device pipeline (for validation)
# ---------------------------------------------------------------------------

def _simulate(x, freqs_cis, mask_cache, wqkv, wo, use_bf16=True):
    """Mirror the device computation in numpy.  Returns (out, debug_dict)."""
    def q_(a):  # quantize
        return a.astype(bf16).astype(np.float32) if use_bf16 else a

    prep = _prep(x, freqs_cis, mask_cache, wqkv, wo)
    cls = prep['cls']
    xT = prep['xT'].astype(np.float32)
    ropeA = np.concatenate([prep['ropeA'].astype(np.float32)] * B, axis=1)
    ropeB = np.concatenate([prep['ropeB'].astype(np.float32)] * B, axis=1)
    mask = np.asarray(mask_cache)[0, 0]

    dbg = {c: {} for c in range(NCORES)}
    a2a_ins = {m: [] for m in range(HPC)}  # m -> [core][8*128, 512]
    Dsave = {}
    for c in range(NCORES):
        wT = prep['wTs'][c].astype(np.float32)
        qkvT = q_(wT.T @ xT)       # [768, TOK]  (psum f32, evict to bf16)
        sw = np.empty_like(qkvT[:5 * P])
        for f in range(5):
            blk = qkvT[f * P:(f + 1) * P]
            sw[f * P:(f + 1) * P] = q_(blk[[i ^ 1 for i in range(P)], :])
        roped = np.empty_like(qkvT[:5 * P])
        for f in range(5):
            blk = qkvT[f * P:(f + 1) * P]
            r1 = q_(blk * ropeA)
            r2 = q_(sw[f * P:(f + 1) * P] * ropeB)
            roped[f * P:(f + 1) * P] = q_(r1 + r2)
        qTs = [roped[h * P:(h + 1) * P] for h in range(HPC)]
        kTc = roped[4 * P:5 * P]
        vT = qkvT[5 * P:6 * P]     # [128 d, TOK], not roped
        dbg[c]['qT0'] = qTs[0]
        dbg[c]['kT'] = kTc
        dbg[c]['v'] = vT           # device dumps v chunks transposed back
        for h in range(HPC):
            a2a_c = np.zeros((NCORES * P, 512), dtype=np.float32)
            for b in range(B):
                kTb = kTc[:, b * S:(b + 1) * S]
                vTb = vT[:, b * S:(b + 1) * S]
                qTb = qTs[h][:, b * S:(b + 1) * S]
                sT = kTb.T @ qTb               # [Sk, Sq] psum f32
                e = q_(np.exp(sT * SCALE))     # ACT exp -> bf16
                emask = e * mask.T             # mask multiply (exact 0/1)
                # zero out skipped tiles entirely
                for qt in range(QT_N):
                    for kt in range(KT_N):
                        if cls[qt][kt] == 'skip':
                            emask[kt * P:(kt + 1) * P,
                                  qt * 512:(qt + 1) * 512] = 0
                D = emask.sum(axis=0)          # psum f32
                rec = 1.0 / D
                # y_u[d, q] = sum_k v[k, d] e[k, q];  vTb is [d, k]
                yTu = vTb @ emask
                y = q_(yTu * rec[None, :])
                for qt in range(QT_N):
                    r = b * QT_N + qt
                    a2a_c[r * P:(r + 1) * P] = y[:, qt * 512:(qt + 1) * 512]
                Dsave[(c, h, b)] = D
            a2a_ins[h].append(a2a_c)
        dbg[c]['a2ai0'] = a2a_ins[0][c]

    # route the A2As:  out shard j on rank c = rank j's input shard c
    out_full = np.zeros((B, S, DIM), dtype=np.float32)
    woT = prep['woT'].astype(np.float32)
    for c in range(NCORES):
        yfull = np.zeros((DIM, 512), dtype=np.float32)
        for m in range(HPC):
            for j in range(NCORES):
                dbi = m * NCORES + j
                yfull[dbi * P:(dbi + 1) * P] = \
                    a2a_ins[m][j][c * P:(c + 1) * P]
        dbg[c]['a2ao0'] = yfull[:NCORES * P]
        o = yfull.T @ woT          # [512 tok, DIM] psum f32
        b, jj = c // QT_N, c % QT_N
        out_full[b, jj * 512:(jj + 1) * 512] = o
    return out_full, dbg



# revision 28
# speedup vs baseline: 1.0754x; 1.0754x over previous
"""Distributed Trainium2 kernel for nn_Attention_37958920962105.

GQA attention layer (DIM=4096, 32 q heads, 8 kv heads, head_dim=128,
B=2, S=2048) sharded tensor-parallel over GQA groups across 8 cores:
core c owns q heads 4c..4c+3 and kv head c.  Per core:
  1. QKV projection (transposed layouts) + RoPE + v transpose
  2. attention (scores -> exp -> ones-matmul denominators -> AV)
  3. AllToAll (one per local head) to token-shard y
  4. out projection on this core's 512-token chunk
Output chunks are reassembled on the host.
"""

import math
import sys
import types
from contextlib import ExitStack

import numpy as np
import ml_dtypes

import concourse.bass as bass
import concourse.mybir as mybir
import concourse.tile as tile
from concourse import bacc
from concourse.bass_utils import run_bass_kernel_spmd

BF = mybir.dt.bfloat16
F32 = mybir.dt.float32
bf16 = ml_dtypes.bfloat16

P = 128
DIM = 4096
N_HEAD = 32
N_KV = 8
HD = 128
B = 2
S = 2048
TOK = B * S          # 4096
NCORES = 8
HPC = N_HEAD // N_KV  # 4 q heads per core
FQKV = (HPC + 2) * HD  # 768 qkv rows per core
KC = DIM // P        # 32 contraction chunks
N_TT = TOK // 512    # 8 token tiles of 512
QT_N = S // 512      # 4 q tiles per batch
KT_N = S // P        # 16 k tiles per batch
SCALE = 1.0 / math.sqrt(HD)


def _install_profile_shim():
    if 'antenv.axon_hooks' in sys.modules:
        return
    try:
        from trn_agent_boot.trn_boot import _ntff_profile_via_ctypes
        hook = _ntff_profile_via_ctypes('/opt/axon/libaxon_pjrt.so')
    except Exception:
        hook = None
    mod = types.ModuleType('antenv.axon_hooks')
    mod._hook = hook
    mod.get_axon_ntff_profile_hook = lambda: mod._hook
    mod.set_axon_ntff_profile_hook = lambda h: setattr(mod, '_hook', h)
    sys.modules['antenv.axon_hooks'] = mod
    try:
        import antenv
        antenv.axon_hooks = mod
    except ImportError:
        pass


# ---------------------------------------------------------------------------
# host-side prep
# ---------------------------------------------------------------------------

def _classify_mask(mask):
    """mask: [S(q), S(k)] bool.  Returns (cls, mask_tiles) where
    cls[qt][kt] in {'skip', 'free', int mask-tile-index}; mask tiles are
    transposed [128 k, 512 q] bf16."""
    cls = [[None] * KT_N for _ in range(QT_N)]
    tiles = []
    seen = {}
    qi = np.arange(512)[:, None]
    ki = np.arange(P)[None, :]
    for qt in range(QT_N):
        for kt in range(KT_N):
            blk = mask[qt * 512:(qt + 1) * 512, kt * P:(kt + 1) * P]
            if not blk.any():
                cls[qt][kt] = 'skip'
                continue
            if blk.all():
                cls[qt][kt] = 'free'
                continue
            off = kt * P - qt * 512
            if 0 <= off < 512 and np.array_equal(blk, (off + ki) <= qi):
                # causal diagonal tile: valid only for q >= off, and within
                # the first 128 valid q columns it is the k<=q' triangle
                cls[qt][kt] = ('diag', off)
                continue
            key = blk.tobytes()
            if key not in seen:
                seen[key] = len(tiles)
                tiles.append(np.ascontiguousarray(blk.T).astype(bf16))
            cls[qt][kt] = seen[key]
    return cls, tiles


def _prep(x, freqs_cis, mask_cache, wqkv, wo):
    x = np.asarray(x, dtype=np.float32)
    freqs_cis = np.asarray(freqs_cis, dtype=np.float32)
    wqkv = np.asarray(wqkv, dtype=np.float32)
    wo = np.asarray(wo, dtype=np.float32)
    mask = np.asarray(mask_cache)[0, 0]

    xT = np.ascontiguousarray(x.reshape(TOK, DIM).T).astype(bf16)

    wTs = []
    for c in range(NCORES):
        w_c = np.concatenate([
            wqkv[HPC * HD * c: HPC * HD * (c + 1)],          # 4 q heads
            wqkv[N_HEAD * HD + HD * c: N_HEAD * HD + HD * (c + 1)],   # k head
            wqkv[(N_HEAD + N_KV) * HD + HD * c:
                 (N_HEAD + N_KV) * HD + HD * (c + 1)],       # v head
        ], axis=0)                                           # [768, DIM]
        wTs.append(np.ascontiguousarray(w_c.T).astype(bf16))  # [DIM, 768]

    # wo permuted so row-block dbi = m*8 + cc holds global head 4*cc + m
    woT = np.ascontiguousarray(wo.T)                 # [d, o]
    woT_h = woT.reshape(N_HEAD, HD, DIM)
    perm = [4 * (dbi % NCORES) + dbi // NCORES for dbi in range(N_HEAD)]
    woT_perm = np.ascontiguousarray(woT_h[perm].reshape(DIM, DIM)).astype(bf16)

    f0 = freqs_cis[:, :, 0].T                        # [64, S]
    f1 = freqs_cis[:, :, 1].T
    ropeA = np.repeat(f0, 2, axis=0).astype(bf16)    # [128, S]
    ropeB = np.empty((HD, S), dtype=np.float32)
    ropeB[0::2] = -f1
    ropeB[1::2] = f1
    ropeB = ropeB.astype(bf16)

    pswap = np.zeros((P, P), dtype=bf16)
    for i in range(P):
        pswap[i, i ^ 1] = 1
    ident = np.eye(P, dtype=bf16)
    ones_col = np.ones((P, 1), dtype=bf16)
    ones_row = np.ones((1, P), dtype=bf16)
    onesc_f32 = np.ones((1, P), dtype=np.float32)

    tri = (np.arange(P)[:, None] <= np.arange(P)[None, :]).astype(bf16)
    cls, mask_tiles = _classify_mask(mask)
    masks = (np.concatenate([t for t in mask_tiles], axis=0)
             if mask_tiles else None)                # [n*128, 512] bf16

    return dict(xT=xT, wTs=wTs, woT=woT_perm, ropeA=ropeA, ropeB=ropeB,
                pswap=pswap, ident=ident, ones=ones_col, onesr=ones_row,
                onesc=onesc_f32, tri=tri, cls=cls, masks=masks)


# ---------------------------------------------------------------------------
# device kernel builder
# ---------------------------------------------------------------------------

def _build(cls, n_masks, debug=False):
    nc = bacc.Bacc("TRN2", target_bir_lowering=False, debug=False,
                   num_devices=NCORES)
    xT_d = nc.dram_tensor("xT", [DIM, TOK], BF, kind="ExternalInput")
    wT_d = nc.dram_tensor("wT", [DIM, FQKV], BF, kind="ExternalInput")
    woT_d = nc.dram_tensor("woT", [DIM, DIM], BF, kind="ExternalInput")
    ropeA_d = nc.dram_tensor("ropeA", [P, S], BF, kind="ExternalInput")
    ropeB_d = nc.dram_tensor("ropeB", [P, S], BF, kind="ExternalInput")
    pswap_d = nc.dram_tensor("pswap", [P, P], BF, kind="ExternalInput")
    ident_d = nc.dram_tensor("ident", [P, P], BF, kind="ExternalInput")
    ones_d = nc.dram_tensor("ones", [P, 1], BF, kind="ExternalInput")
    onesr_d = nc.dram_tensor("onesr", [1, P], BF, kind="ExternalInput")
    tri_d = nc.dram_tensor("tri", [P, P], BF, kind="ExternalInput")
    masks_d = (nc.dram_tensor("masks", [n_masks * P, 512], BF,
                              kind="ExternalInput") if n_masks else None)
    out_d = nc.dram_tensor("out", [512, DIM], BF, kind="ExternalOutput")
    if debug:
        dbg_qT0 = nc.dram_tensor("dbg_qT0", [P, TOK], BF, kind="ExternalOutput")
        dbg_kT = nc.dram_tensor("dbg_kT", [P, TOK], BF, kind="ExternalOutput")
        dbg_v = nc.dram_tensor("dbg_v", [P, TOK], BF, kind="ExternalOutput")
        dbg_a2ai = nc.dram_tensor("dbg_a2ai", [8 * P, 512], BF, kind="ExternalOutput")
        dbg_a2ao = nc.dram_tensor("dbg_a2ao", [8 * P, 512], BF, kind="ExternalOutput")

    EXP = mybir.ActivationFunctionType.Exp
    rg = [list(range(NCORES))]

    with tile.TileContext(nc) as tc:
        with ExitStack() as top:
            const = top.enter_context(tc.tile_pool(name="const", bufs=1))
            acts = top.enter_context(tc.tile_pool(name="acts", bufs=1))
            dramp = top.enter_context(tc.tile_pool(name="dramp", bufs=1,
                                                   space="DRAM"))

            qT = [acts.tile([P, TOK], BF, name=f"qT{h}") for h in range(HPC)]
            kT = acts.tile([P, TOK], BF, name="kT")
            vv = [acts.tile([P, P], BF, name=f"v{i}") for i in range(TOK // P)]
            yf = [acts.tile([P, 512], BF, name=f"yf{i}") for i in range(KC)]

            a2a_in = [dramp.tile([NCORES * P, 512], BF, name=f"a2ai{m}")
                      for m in range(HPC)]
            a2a_out = [dramp.tile([NCORES * P, 512], BF, name=f"a2ao{m}")
                       for m in range(HPC)]

            # ---------------- phase 1: QKV + rope + v transpose ----------
            with ExitStack() as ph1, nc.named_scope("ph1_qkv"):
                wp = ph1.enter_context(tc.tile_pool(name="wp", bufs=1))
                xp = ph1.enter_context(tc.tile_pool(name="xp", bufs=48))
                w_sb = []
                xts0 = []
                for kc in range(KC):
                    w = wp.tile([P, FQKV], BF, name=f"w{kc}")
                    (nc.sync if kc % 2 else nc.scalar).dma_start(
                        w[:], wT_d[kc * P:(kc + 1) * P, :])
                    w_sb.append(w)
                    xt = xp.tile([P, 512], BF, name="xt")
                    (nc.scalar if kc % 2 else nc.sync).dma_start(
                        xt[:], xT_d[kc * P:(kc + 1) * P, 0:512])
                    xts0.append(xt)

                ropeA_sb = const.tile([P, S], BF, name="ropeA_sb")
                nc.sync.dma_start(ropeA_sb[:], ropeA_d[:])
                ropeB_sb = const.tile([P, S], BF, name="ropeB_sb")
                nc.sync.dma_start(ropeB_sb[:], ropeB_d[:])
                pswap_sb = const.tile([P, P], BF, name="pswap_sb")
                nc.sync.dma_start(pswap_sb[:], pswap_d[:])
                ident_sb = const.tile([P, P], BF, name="ident_sb")
                nc.sync.dma_start(ident_sb[:], ident_d[:])
                ones_sb = const.tile([P, 1], BF, name="ones_sb")
                nc.sync.dma_start(ones_sb[:], ones_d[:])
                onesr_sb = const.tile([1, P], BF, name="onesr_sb")
                nc.sync.dma_start(onesr_sb[:], onesr_d[:])
                tri_sb = const.tile([P, P], BF, name="tri_sb")
                nc.sync.dma_start(tri_sb[:], tri_d[:])
                mask_sb = []
                for i in range(n_masks):
                    m = const.tile([P, 512], BF, name=f"mask{i}")
                    nc.sync.dma_start(m[:], masks_d[i * P:(i + 1) * P, :])
                    mask_sb.append(m)
                qkvp = ph1.enter_context(
                    tc.tile_pool(name="qkvp", bufs=6, space="PSUM"))
                miscp = ph1.enter_context(
                    tc.tile_pool(name="miscp", bufs=2, space="PSUM"))
                stg = ph1.enter_context(tc.tile_pool(name="stg", bufs=4))

                for tt in range(N_TT):
                    s0 = (tt % QT_N) * 512
                    if tt == 0:
                        xts = xts0
                    else:
                        xts = []
                        for kc in range(KC):
                            xt = xp.tile([P, 512], BF, name="xt")
                            (nc.scalar if kc % 2 else nc.sync).dma_start(
                                xt[:], xT_d[kc * P:(kc + 1) * P,
                                            tt * 512:(tt + 1) * 512])
                            xts.append(xt)
                    pss = [qkvp.tile([P, 512], F32, name="qkvps")
                           for _ in range(6)]
                    for kc in range(KC):
                        for f in range(6):
                            nc.tensor.matmul(
                                pss[f][:], w_sb[kc][:, f * P:(f + 1) * P],
                                xts[kc][:], start=(kc == 0),
                                stop=(kc == KC - 1))
                    for f in range(6):
                        raw = stg.tile([P, 512], BF, name="raw")
                        nc.scalar.copy(raw[:], pss[f][:])
                        if f < 5:
                            swp = miscp.tile([P, 512], F32, name="miscps")
                            nc.tensor.matmul(swp[:], pswap_sb[:], raw[:],
                                             start=True, stop=True)
                            sw_sb = stg.tile([P, 512], BF, name="sw_sb")
                            nc.scalar.copy(sw_sb[:], swp[:])
                            r1 = stg.tile([P, 512], BF, name="r1")
                            nc.vector.tensor_mul(r1[:], raw[:],
                                                 ropeA_sb[:, s0:s0 + 512])
                            r2 = stg.tile([P, 512], BF, name="r2")
                            nc.vector.tensor_mul(r2[:], sw_sb[:],
                                                 ropeB_sb[:, s0:s0 + 512])
                            dst = (qT[f] if f < HPC else kT)
                            nc.vector.tensor_add(
                                dst[:, tt * 512:(tt + 1) * 512], r1[:], r2[:])
                        else:
                            for j in range(4):
                                tp = miscp.tile([P, 512], BF, name="miscps")
                                nc.tensor.transpose(
                                    tp[:, :P], raw[:, j * P:(j + 1) * P],
                                    ident_sb[:])
                                nc.vector.tensor_copy(vv[tt * 4 + j][:], tp[:, :P])

            # wo prefetch pool opened early so its DMAs (scalar ring) can
            # stream during attention
            wop = top.enter_context(tc.tile_pool(name="wop", bufs=2))
            wo_tiles = {}

            def wo_prefetch(ot, eng=None):
                t = wop.tile([P, KC, 512], BF, name="wo_sb")
                for dc in range(KC):
                    e = eng or (nc.scalar if dc % 2 else nc.sync)
                    e.dma_start(
                        t[:, dc, :],
                        woT_d[dc * P:(dc + 1) * P, ot * 512:(ot + 1) * 512])
                wo_tiles[ot] = t

            wo_prefetch(0, nc.sync)
            wo_prefetch(1, nc.sync)

            # ---------------- phase 2: attention + A2A -------------------
            with ExitStack() as ph2, nc.named_scope("ph2_attn"):
                sp = ph2.enter_context(
                    tc.tile_pool(name="sp", bufs=4, space="PSUM"))
                yp = ph2.enter_context(
                    tc.tile_pool(name="yp", bufs=2, space="PSUM"))
                dp = ph2.enter_context(
                    tc.tile_pool(name="dp", bufs=1, space="PSUM"))
                bp = ph2.enter_context(
                    tc.tile_pool(name="bp", bufs=1, space="PSUM"))
                ep = ph2.enter_context(tc.tile_pool(name="ep", bufs=6))
                eaccp = ph2.enter_context(tc.tile_pool(name="eaccp", bufs=2))
                ys = ph2.enter_context(tc.tile_pool(name="ys", bufs=5))
                rp = ph2.enter_context(tc.tile_pool(name="rp", bufs=4))

                def finish_norm(h, b, qt, yu_sb, e_acc):
                    # D = col-sums of e_acc via ones-matmul; broadcast 1/D
                    # across partitions with a rank-1 matmul (no DRAM trip)
                    ps_d = dp.tile([1, 512], F32, name="psd")
                    nc.tensor.matmul(ps_d[:], ones_sb[:], e_acc[:],
                                     start=True, stop=True)
                    rec = rp.tile([1, 512], F32, name="rec")
                    nc.vector.reciprocal_approx_fast(rec[:], ps_d[:])
                    rec_b = rp.tile([1, 512], BF, name="rec_b")
                    nc.vector.tensor_copy(rec_b[:], rec[:])
                    ps_b = bp.tile([P, 512], F32, name="psb")
                    nc.tensor.matmul(ps_b[:], onesr_sb[:], rec_b[:],
                                     start=True, stop=True)
                    y_sb = ys.tile([P, 512], BF, name="y_sb")
                    nc.vector.tensor_mul(y_sb[:], yu_sb[:], ps_b[:])
                    r = b * QT_N + qt
                    nc.sync.dma_start(
                        a2a_in[h][r * P:(r + 1) * P, :], y_sb[:])

                yf_pending = []

                def flush_yf():
                    while yf_pending:
                        hh = yf_pending.pop(0)
                        for cc in range(NCORES):
                            nc.gpsimd.dma_start(
                                yf[hh * NCORES + cc][:],
                                a2a_out[hh][cc * P:(cc + 1) * P, :])

                for h in range(HPC):
                    pending = []
                    flush_yf()
                    for b in range(B):
                        for qt in range(QT_N):
                            kts = [kt for kt in range(KT_N)
                                   if cls[qt][kt] != 'skip']
                            ps_y = yp.tile([P, 512], F32, name="psy")
                            e_acc = eaccp.tile([P, 512], BF, name="eacc")
                            for i, kt in enumerate(kts):
                                c = cls[qt][kt]
                                off = 0
                                tri_mask = None
                                if isinstance(c, tuple):
                                    off = c[1]
                                    tri_mask = tri_sb
                                w_q = 512 - off
                                q0 = b * S + qt * 512 + off
                                ps_s = sp.tile([P, 512], F32, name="pss")
                                nc.tensor.matmul(
                                    ps_s[:, :w_q],
                                    kT[:, b * S + kt * P: b * S + (kt + 1) * P],
                                    qT[h][:, bass.ds(q0, w_q)],
                                    start=True, stop=True)
                                # first tile (always full width) exps straight
                                # into the accumulator; later tiles add on DVE
                                if i == 0:
                                    e = e_acc
                                else:
                                    e = ep.tile([P, w_q], BF, name="e")
                                nc.scalar.activation(e[:], ps_s[:, :w_q], EXP,
                                                     scale=SCALE)
                                if tri_mask is not None:
                                    nc.vector.tensor_mul(
                                        e[:, :P], e[:, :P], tri_mask[:])
                                elif c != 'free':
                                    em = ep.tile([P, 512], BF, name="em")
                                    nc.vector.tensor_mul(
                                        em[:], e[:], mask_sb[c][:])
                                    e = em
                                if i > 0:
                                    nc.vector.tensor_add(
                                        e_acc[:, off:512],
                                        e_acc[:, off:512], e[:, :w_q])
                                st, sf = (i == 0), (i == len(kts) - 1)
                                nc.tensor.matmul(ps_y[:, off:512],
                                                 vv[b * KT_N + kt][:],
                                                 e[:], start=st, stop=sf,
                                                 skip_group_check=True)
                            yu_sb = ys.tile([P, 512], F32, name="yu_sb")
                            nc.vector.tensor_copy(yu_sb[:], ps_y[:])
                            pending.append((h, b, qt, yu_sb, e_acc))
                            if len(pending) > 1:
                                finish_norm(*pending.pop(0))
                    for pn in pending:
                        finish_norm(*pn)
                    pending = []
                    nc.gpsimd.collective_compute(
                        "AllToAll", mybir.AluOpType.bypass,
                        replica_groups=rg,
                        ins=[a2a_in[h].opt()], outs=[a2a_out[h].opt()])
                    yf_pending.append(h)
                flush_yf()

            # ---------------- phase 3: out projection --------------------
            with ExitStack() as ph3, nc.named_scope("ph3_outp"):
                opp = ph3.enter_context(
                    tc.tile_pool(name="opp", bufs=4, space="PSUM"))
                osb = ph3.enter_context(tc.tile_pool(name="osb", bufs=3))
                for ot in range(8):
                    wo_sb = wo_tiles.pop(ot)
                    if ot + 2 < 8:
                        wo_prefetch(ot + 2)
                    if ot == 0:
                        # first o-tile: run the first 24 d-chunks for every
                        # token subtile before touching m=3 blocks, so the
                        # last AllToAll's latency hides under real work
                        psos = [opp.tile([P, 512], F32, name="pso")
                                for _ in range(4)]
                        for ts in range(4):
                            for dc in range(24):
                                nc.tensor.matmul(
                                    psos[ts][:], yf[dc][:, ts * P:(ts + 1) * P],
                                    wo_sb[:, dc, :], start=(dc == 0),
                                    stop=False, skip_group_check=True)
                        for ts in range(4):
                            for dc in range(24, KC):
                                nc.tensor.matmul(
                                    psos[ts][:], yf[dc][:, ts * P:(ts + 1) * P],
                                    wo_sb[:, dc, :], start=False,
                                    stop=(dc == KC - 1), skip_group_check=True)
                            ob = osb.tile([P, 512], BF, name="ob")
                            nc.scalar.copy(ob[:], psos[ts][:])
                            nc.sync.dma_start(
                                out_d[ts * P:(ts + 1) * P, 0:512], ob[:])
                        continue
                    for ts in range(4):
                        pso = opp.tile([P, 512], F32, name="pso")
                        for dc in range(KC):
                            nc.tensor.matmul(
                                pso[:], yf[dc][:, ts * P:(ts + 1) * P],
                                wo_sb[:, dc, :], start=(dc == 0),
                                stop=(dc == KC - 1))
                        ob = osb.tile([P, 512], BF, name="ob")
                        nc.scalar.copy(ob[:], pso[:])
                        nc.sync.dma_start(
                            out_d[ts * P:(ts + 1) * P,
                                  ot * 512:(ot + 1) * 512], ob[:])

            if debug:
                nc.sync.dma_start(dbg_qT0[:], qT[0][:])
                nc.sync.dma_start(dbg_kT[:], kT[:])
                for i in range(TOK // P):
                    nc.sync.dma_start(dbg_v[:, i * P:(i + 1) * P], vv[i][:])
                nc.sync.dma_start(dbg_a2ai[:], a2a_in[0][:])
                nc.sync.dma_start(dbg_a2ao[:], a2a_out[0][:])

    nc.compile()
    return nc


# ---------------------------------------------------------------------------
# public entry
# ---------------------------------------------------------------------------

_CACHE = {}


def _execute(x, freqs_cis, mask_cache, input_pos, wqkv, wo,
             trace=False, debug=False):
    _install_profile_shim()
    prep = _prep(x, freqs_cis, mask_cache, wqkv, wo)
    cls = prep['cls']
    n_masks = 0 if prep['masks'] is None else prep['masks'].shape[0] // P
    key = (str(cls), n_masks, debug)
    if key not in _CACHE:
        _CACHE[key] = _build(cls, n_masks, debug=debug)
    nc = _CACHE[key]

    in_maps = []
    for c in range(NCORES):
        m = dict(xT=prep['xT'], wT=prep['wTs'][c], woT=prep['woT'],
                 ropeA=prep['ropeA'], ropeB=prep['ropeB'],
                 pswap=prep['pswap'], ident=prep['ident'],
                 ones=prep['ones'], onesr=prep['onesr'], tri=prep['tri'])
        if n_masks:
            m['masks'] = prep['masks']
        in_maps.append(m)

    res = run_bass_kernel_spmd(nc, in_maps, core_ids=list(range(NCORES)),
                               trace=trace,
                               trace_cores=list(range(NCORES)) if trace
                               else None)
    out = np.zeros((B, S, DIM), dtype=np.float32)
    for c in range(NCORES):
        b, j = c // QT_N, c % QT_N
        out[b, j * 512:(j + 1) * 512] = \
            res.results[c]['out'].astype(np.float32)
    return out, res


def kernel(x, freqs_cis, mask_cache, input_pos, wqkv, wo):
    out, _ = _execute(x, freqs_cis, mask_cache, input_pos, wqkv, wo)
    return out


# ---------------------------------------------------------------------------
# numpy simulation of the exact device pipeline (for validation)
# ---------------------------------------------------------------------------

def _simulate(x, freqs_cis, mask_cache, wqkv, wo, use_bf16=True):
    """Mirror the device computation in numpy.  Returns (out, debug_dict)."""
    def q_(a):  # quantize
        return a.astype(bf16).astype(np.float32) if use_bf16 else a

    prep = _prep(x, freqs_cis, mask_cache, wqkv, wo)
    cls = prep['cls']
    xT = prep['xT'].astype(np.float32)
    ropeA = np.concatenate([prep['ropeA'].astype(np.float32)] * B, axis=1)
    ropeB = np.concatenate([prep['ropeB'].astype(np.float32)] * B, axis=1)
    mask = np.asarray(mask_cache)[0, 0]

    dbg = {c: {} for c in range(NCORES)}
    a2a_ins = {m: [] for m in range(HPC)}  # m -> [core][8*128, 512]
    Dsave = {}
    for c in range(NCORES):
        wT = prep['wTs'][c].astype(np.float32)
        qkvT = q_(wT.T @ xT)       # [768, TOK]  (psum f32, evict to bf16)
        sw = np.empty_like(qkvT[:5 * P])
        for f in range(5):
            blk = qkvT[f * P:(f + 1) * P]
            sw[f * P:(f + 1) * P] = q_(blk[[i ^ 1 for i in range(P)], :])
        roped = np.empty_like(qkvT[:5 * P])
        for f in range(5):
            blk = qkvT[f * P:(f + 1) * P]
            r1 = q_(blk * ropeA)
            r2 = q_(sw[f * P:(f + 1) * P] * ropeB)
            roped[f * P:(f + 1) * P] = q_(r1 + r2)
        qTs = [roped[h * P:(h + 1) * P] for h in range(HPC)]
        kTc = roped[4 * P:5 * P]
        vT = qkvT[5 * P:6 * P]     # [128 d, TOK], not roped
        dbg[c]['qT0'] = qTs[0]
        dbg[c]['kT'] = kTc
        dbg[c]['v'] = vT           # device dumps v chunks transposed back
        for h in range(HPC):
            a2a_c = np.zeros((NCORES * P, 512), dtype=np.float32)
            for b in range(B):
                kTb = kTc[:, b * S:(b + 1) * S]
                vTb = vT[:, b * S:(b + 1) * S]
                qTb = qTs[h][:, b * S:(b + 1) * S]
                sT = kTb.T @ qTb               # [Sk, Sq] psum f32
                e = q_(np.exp(sT * SCALE))     # ACT exp -> bf16
                emask = e * mask.T             # mask multiply (exact 0/1)
                # zero out skipped tiles entirely
                for qt in range(QT_N):
                    for kt in range(KT_N):
                        if cls[qt][kt] == 'skip':
                            emask[kt * P:(kt + 1) * P,
                                  qt * 512:(qt + 1) * 512] = 0
                D = emask.sum(axis=0)          # psum f32
                rec = 1.0 / D
                # y_u[d, q] = sum_k v[k, d] e[k, q];  vTb is [d, k]
                yTu = vTb @ emask
                y = q_(yTu * rec[None, :])
                for qt in range(QT_N):
                    r = b * QT_N + qt
                    a2a_c[r * P:(r + 1) * P] = y[:, qt * 512:(qt + 1) * 512]
                Dsave[(c, h, b)] = D
            a2a_ins[h].append(a2a_c)
        dbg[c]['a2ai0'] = a2a_ins[0][c]

    # route the A2As:  out shard j on rank c = rank j's input shard c
    out_full = np.zeros((B, S, DIM), dtype=np.float32)
    woT = prep['woT'].astype(np.float32)
    for c in range(NCORES):
        yfull = np.zeros((DIM, 512), dtype=np.float32)
        for m in range(HPC):
            for j in range(NCORES):
                dbi = m * NCORES + j
                yfull[dbi * P:(dbi + 1) * P] = \
                    a2a_ins[m][j][c * P:(c + 1) * P]
        dbg[c]['a2ao0'] = yfull[:NCORES * P]
        o = yfull.T @ woT          # [512 tok, DIM] psum f32
        b, jj = c // QT_N, c % QT_N
        out_full[b, jj * 512:(jj + 1) * 512] = o
    return out_full, dbg

